# revision 13
# baseline (speedup 1.0000x reference)
"""Octonion-structured causal self-attention on 8 Trainium2 NeuronCores.

Strategy (2 SPMD launches, on-device AllGathers to minimize host->device
traffic; tensor-parallel over output-channel blocks in both launches):

  Launch 1 -- each core uploads only its 512-channel slice of x^T (2MB);
    an on-device 8-way AllGather rebuilds the full x^T [4096,2048] in HBM.
    Each core then computes q^T,k^T (RoPE'd, channel-pair-permuted) and v
    for its 512-channel block (= its 4 heads), then causal attention for
    those heads, producing softmax-normalized y^T [512, 2048].

  Launch 2 -- the octonion head-mixer is folded into Wo on the HOST
    (Wcomb[g-rows,:] = M_g^T @ Wo_eff[g-rows,:], cached across calls), so
    the device only runs one GEMM: out = y_att^T . Wcomb. Each core uploads
    its own y^T slice (2MB, exactly what launch 1 returned) + its 512
    output columns of Wcomb (4MB); an on-device AllGather rebuilds the full
    y_att [4096, 2048]; each core computes out^T[:, :] for its columns.

All matmuls run in bf16 (TensorE full rate); accumulation is fp32 in PSUM.
RoPE trick: channels of q/k are permuted host-side (per head: evens then
odds) by permuting W_q/W_k columns, so the rotation pairs become the two
partition halves of each head tile; scores are invariant to a shared q/k
channel permutation, and v/y stay in natural order.

Softmax row-sums are accumulated on the Vector engine (f32) and reduced
with a single ones-matmul per (head, query-block) instead of one matmul
per key-chunk, keeping TensorE for real FLOPs.
"""
import json
import math
import sys

sys.path.insert(0, '/opt/trn_rl_repo')

import numpy as np
import ml_dtypes

import concourse.bass as bass
import concourse.mybir as mybir
import concourse.tile as tile

F32 = mybir.dt.float32
BF16 = mybir.dt.bfloat16
BF = ml_dtypes.bfloat16

B, T, C, H, D = 1, 2048, 4096, 32, 128
NC = 8            # cores
CPB = C // NC     # channels per core (512) = 4 heads
TBLK = 512        # projection T-block
TQB = 512         # attention query block
NTB = T // TBLK   # 4
INV_SQRT_D = 1.0 / math.sqrt(D)

# ---------------------------------------------------------------- walrus fix
# This container's walrus encodes at most ONE sync-wait per instruction;
# Tile attaches several. Split extras into single-wait NoOps just before the
# instruction (same engine => same program point; semantics unchanged).
_ws_counter = [0]


def _split_multiwaits_json(bir_bytes):
    m = json.loads(bir_bytes)
    changed_any = False
    for fn in m.get("functions", []):
        for blk in fn.get("blocks", []):
            insts = blk.get("instructions")
            if not insts:
                continue
            out, changed = [], False
            for inst in insts:
                si = inst.get("sync_info")
                waits = si.get("on_wait") if si else None
                if waits and len(waits) > 1:
                    changed = True
                    for w in waits[:-1]:
                        _ws_counter[0] += 1
                        out.append({
                            "engine": inst["engine"], "ins": [], "outs": [],
                            "name": f"I-wsplit-{_ws_counter[0]}",
                            "opcode": "NoOp",
                            "sync_info": {"on_wait": [w], "on_update": []},
                        })
                    si["on_wait"] = [waits[-1]]
                out.append(inst)
            if changed:
                blk["instructions"] = out
                changed_any = True
    return json.dumps(m).encode() if changed_any else bir_bytes


_patched = [False]


def _install_patch():
    if _patched[0]:
        return
    _patched[0] = True
    import concourse.bass_utils as bass_utils
    import concourse.bass2jax as bass2jax

    orig = bass_utils.compile_bir_kernel

    def patched(bir_json, tmpdir, neff_name="file.neff"):
        if isinstance(bir_json, str):
            bir_json = bir_json.encode()
        return orig(_split_multiwaits_json(bir_json), tmpdir, neff_name=neff_name)

    bass_utils.compile_bir_kernel = patched
    bass2jax.compile_bir_kernel = patched


# ------------------------------------------------------- octonion structure
def _cd_tables(levels=3):
    idx = np.array([[0]])
    sgn = np.array([[1]])
    for _ in range(levels):
        n = idx.shape[0]
        N2 = 2 * n
        I = np.zeros((N2, N2), np.int64)
        S = np.zeros((N2, N2), np.int64)
        cj = lambda j: 1 if j == 0 else -1
        for i in range(n):
            for j in range(n):
                I[i, j] = idx[i, j]
                S[i, j] = sgn[i, j]
                I[i, n + j] = n + idx[j, i]
                S[i, n + j] = sgn[j, i]
                I[n + i, j] = n + idx[i, j]
                S[n + i, j] = sgn[i, j] * cj(j)
                I[n + i, n + j] = idx[j, i]
                S[n + i, n + j] = -cj(j) * sgn[j, i]
        idx, sgn = I, S
    return idx, sgn


_OIDX, _OSGN = _cd_tables()
_SIGN = np.array([[_OSGN[j, i ^ j] for j in range(8)] for i in range(8)], np.float32)


def _weff(W):
    """[8, 512, 512] -> dense [4096, 4096]: block (row j, col i) = SIGN[i,j]*W[i^j]."""
    out = np.empty((C, C), np.float32)
    for i in range(8):
        for j in range(8):
            out[j * 512:(j + 1) * 512, i * 512:(i + 1) * 512] = _SIGN[i, j] * W[i ^ j]
    return out


# ----------------------------------------------------------- phase-1 kernel
def _build_phase1(reps=1, use_ag=True):
    nc = bass.Bass(trn_type="TRN2")
    xs_d = nc.dram_tensor("xs", [CPB, T], BF16, kind="ExternalInput")
    wq_d = nc.dram_tensor("wq", [512, 4096], BF16, kind="ExternalInput")
    wk_d = nc.dram_tensor("wk", [512, 4096], BF16, kind="ExternalInput")
    wv_d = nc.dram_tensor("wv", [128, 32 * 512], BF16, kind="ExternalInput")
    cs_d = nc.dram_tensor("cs", [128, T], F32, kind="ExternalInput")
    sn_d = nc.dram_tensor("sn", [128, T], F32, kind="ExternalInput")
    mk_d = nc.dram_tensor("mk", [128, 128], BF16, kind="ExternalInput")
    yt_d = nc.dram_tensor("yt", [CPB, T], BF16, kind="ExternalOutput")

    with tile.TileContext(nc) as tc:
        with tc.tile_pool(name="dram", bufs=1, space="DRAM") as dram, \
             tc.tile_pool(name="const", bufs=1) as constp, \
             tc.tile_pool(name="qkv", bufs=1) as qkvp, \
             tc.tile_pool(name="xstg", bufs=1) as xsp, \
             tc.tile_pool(name="xres", bufs=2) as xp, \
             tc.tile_pool(name="wvres", bufs=1) as wvp:

            # ---- x AllGather: xs (my 512 chans) -> xa_i (full x^T in 4 waves)
            xin = dram.tile([CPB, T], BF16, name="xin")
            xa = [dram.tile([NC, 128, T], BF16, addr_space="Shared" if use_ag else "Local",
                            name=f"xa{i}") for i in range(4)]
            for i in range(4):
                xcp = xsp.tile([128, T], BF16, tag="xcp", name=f"xcp{i}")
                nc.sync.dma_start(xcp[:], xs_d[i * 128:(i + 1) * 128, :])
                nc.sync.dma_start(xin[i * 128:(i + 1) * 128, :], xcp[:])
            if use_ag:
                for i in range(4):
                    nc.gpsimd.collective_compute(
                        "AllGather", mybir.AluOpType.bypass,
                        replica_groups=[list(range(NC))],
                        ins=[xin[i * 128:(i + 1) * 128, :].opt()],
                        outs=[xa[i][:].opt()])

            cs_s = constp.tile([128, T], F32, tag="cs")
            nc.sync.dma_start(cs_s[:], cs_d[:])
            sn_s = constp.tile([128, T], F32, tag="sn")
            nc.sync.dma_start(sn_s[:], sn_d[:])
            mk_s = constp.tile([128, 128], BF16, tag="mk")
            nc.sync.dma_start(mk_s[:], mk_d[:])
            ones_col = constp.tile([128, 1], BF16, tag="ones_col")
            nc.any.memset(ones_col[:], 1.0)
            ones_row = constp.tile([1, 128], BF16, tag="ones_row")
            nc.any.memset(ones_row[:], 1.0)

            qt_s = qkvp.tile([128, 4 * T], BF16, tag="qt")   # head h at [:, h*T:]
            kt_s = qkvp.tile([128, 4 * T], BF16, tag="kt")
            v_s = qkvp.tile([128, 16 * 512], BF16, tag="v")  # t-chunk tt at [:, tt*512:]

            wv_s = wvp.tile([128, 32 * 512], BF16, tag="wv")
            nc.sync.dma_start(wv_s[:], wv_d[:])

            for _rep in range(reps):
                # ---- projection + attention, interleaved per head so the
                # ACT-bound attention of head h overlaps the PE-bound q/k
                # projection of heads h+1..3. PSUM: pps2+sps2+yps2+dps1+bps1=8.
                with tc.tile_pool(name="wqk", bufs=2) as wqkp, \
                     tc.tile_pool(name="ppsum", bufs=2, space="PSUM") as pps, \
                     tc.tile_pool(name="ropet", bufs=2) as rtp, \
                     tc.tile_pool(name="spsum", bufs=2, space="PSUM") as sps, \
                     tc.tile_pool(name="ypsum", bufs=2, space="PSUM") as yps, \
                     tc.tile_pool(name="dpsum", bufs=1, space="PSUM") as dps, \
                     tc.tile_pool(name="bpsum", bufs=1, space="PSUM") as bps, \
                     tc.tile_pool(name="ptile", bufs=3) as ptp, \
                     tc.tile_pool(name="acct", bufs=2) as accp, \
                     tc.tile_pool(name="attw", bufs=2) as awp:

                    def rope_epilogue(psq, colb, tb, dst):
                        # q' = q*cos_full + swap(q)*sin_signed ; dst bf16 [128,512]
                        qsw = rtp.tile([128, TBLK], F32, tag="qsw")
                        nc.scalar.copy(qsw[0:64, :], psq[64:128, :])
                        nc.scalar.copy(qsw[64:128, :], psq[0:64, :])
                        t1 = rtp.tile([128, TBLK], F32, tag="t1")
                        nc.vector.tensor_mul(t1[:], psq[:], cs_s[:, tb * TBLK:(tb + 1) * TBLK])
                        nc.vector.tensor_mul(qsw[:], qsw[:], sn_s[:, tb * TBLK:(tb + 1) * TBLK])
                        nc.vector.tensor_add(dst, t1[:], qsw[:])

                    def load_xblk(tb):
                        xblk = xp.tile([128, 32 * TBLK], BF16, tag="x")
                        for cc in range(32):
                            r, i = cc // 4, cc % 4
                            nc.sync.dma_start(
                                xblk[:, cc * TBLK:(cc + 1) * TBLK],
                                xa[i][r, :, tb * TBLK:(tb + 1) * TBLK])
                        return xblk

                    # ---- V pass first (v fully ready before attention starts)
                    for tb in range(NTB):
                        xblk = load_xblk(tb)
                        for t128 in range(4):
                            psv = pps.tile([128, 512], F32, tag="pq", name="psv")
                            for c in range(32):
                                nc.tensor.matmul(
                                    psv[:],
                                    xblk[:, c * TBLK + t128 * 128:c * TBLK + (t128 + 1) * 128],
                                    wv_s[:, c * 512:(c + 1) * 512],
                                    start=(c == 0), stop=(c == 31))
                            tt = tb * 4 + t128
                            nc.scalar.copy(v_s[:, tt * 512:(tt + 1) * 512], psv[:])

                    # ---- Q/K pass, head-outer; attention for head h emitted
                    # right after its projection so it overlaps heads h+1..3.
                    for colb in range(4):
                        wq_s = wqkp.tile([128, 4096], BF16, tag="w")
                        nc.sync.dma_start(wq_s[:], wq_d[colb * 128:(colb + 1) * 128, :])
                        wk_s = wqkp.tile([128, 4096], BF16, tag="w")
                        nc.sync.dma_start(wk_s[:], wk_d[colb * 128:(colb + 1) * 128, :])
                        for tb in range(NTB):
                            xblk = load_xblk(tb)
                            psq = pps.tile([128, TBLK], F32, tag="pq", name="psq")
                            for c in range(32):
                                nc.tensor.matmul(
                                    psq[:], wq_s[:, c * 128:(c + 1) * 128],
                                    xblk[:, c * TBLK:(c + 1) * TBLK],
                                    start=(c == 0), stop=(c == 31))
                            rope_epilogue(psq, colb, tb,
                                          qt_s[:, colb * T + tb * TBLK:colb * T + (tb + 1) * TBLK])

                            psk = pps.tile([128, TBLK], F32, tag="pq", name="psk")
                            for c in range(32):
                                nc.tensor.matmul(
                                    psk[:], wk_s[:, c * 128:(c + 1) * 128],
                                    xblk[:, c * TBLK:(c + 1) * TBLK],
                                    start=(c == 0), stop=(c == 31))
                            rope_epilogue(psk, colb, tb,
                                          kt_s[:, colb * T + tb * TBLK:colb * T + (tb + 1) * TBLK])

                        # ---- attention for head h = colb
                        h = colb
                        for tqb in range(4):
                            ntk = (tqb + 1) * 4
                            psy = yps.tile([128, TQB], F32, tag="y")
                            acc = accp.tile([128, TQB], F32, tag="acc")
                            for tkb in range(ntk):
                                di = tkb - tqb * 4
                                pss = sps.tile([128, TQB], F32, tag="s")
                                pt = ptp.tile([128, TQB], BF16, tag="p")
                                if di < 0:
                                    # fully-causal chunk: dense 128x512 scores
                                    nc.tensor.matmul(
                                        pss[:],
                                        kt_s[:, h * T + tkb * 128:h * T + (tkb + 1) * 128],
                                        qt_s[:, h * T + tqb * TQB:h * T + (tqb + 1) * TQB],
                                        start=True, stop=True)
                                    nc.scalar.activation(pt[:], pss[:],
                                                         mybir.ActivationFunctionType.Exp,
                                                         scale=INV_SQRT_D)
                                else:
                                    # diagonal chunk: queries < di*128 are all
                                    # masked; only compute tq in [di*128, 512).
                                    off = di * 128
                                    if off > 0:
                                        nc.vector.memset(pt[:, 0:off], 0.0)
                                    nc.tensor.matmul(
                                        pss[:, off:TQB],
                                        kt_s[:, h * T + tkb * 128:h * T + (tkb + 1) * 128],
                                        qt_s[:, h * T + tqb * TQB + off:h * T + (tqb + 1) * TQB],
                                        start=True, stop=True)
                                    nc.scalar.activation(pt[:, off:TQB], pss[:, off:TQB],
                                                         mybir.ActivationFunctionType.Exp,
                                                         scale=INV_SQRT_D)
                                    # triangular mask on the 128-wide diagonal
                                    nc.vector.tensor_mul(pt[:, off:off + 128],
                                                         pt[:, off:off + 128], mk_s[:])
                                if tkb == 0:
                                    nc.vector.tensor_copy(acc[:], pt[:])
                                else:
                                    nc.vector.tensor_add(acc[:], acc[:], pt[:])
                                nc.tensor.matmul(
                                    psy[:],
                                    v_s[:, tkb * 512 + h * 128:tkb * 512 + (h + 1) * 128],
                                    pt[:],
                                    start=(tkb == 0), stop=(tkb == ntk - 1))
                            accb = awp.tile([128, TQB], BF16, tag="accb")
                            nc.vector.tensor_copy(accb[:], acc[:])
                            psd = dps.tile([1, TQB], F32, tag="d")
                            nc.tensor.matmul(psd[:], ones_col[:], accb[:],
                                             start=True, stop=True)
                            rec = awp.tile([1, TQB], F32, tag="rec")
                            nc.vector.reciprocal(rec[:], psd[:])
                            recb = awp.tile([1, TQB], BF16, tag="recb")
                            nc.vector.tensor_copy(recb[:], rec[:])
                            psb = bps.tile([128, TQB], F32, tag="b")
                            nc.tensor.matmul(psb[:], ones_row[:], recb[:],
                                             start=True, stop=True)
                            recf = awp.tile([128, TQB], BF16, tag="recf")
                            nc.scalar.copy(recf[:], psb[:])
                            ynorm = awp.tile([128, TQB], BF16, tag="yn")
                            nc.vector.tensor_mul(ynorm[:], psy[:], recf[:])
                            nc.sync.dma_start(
                                yt_d[h * 128:(h + 1) * 128, tqb * TQB:(tqb + 1) * TQB],
                                ynorm[:])
    return nc


# ----------------------------------------------------------- phase-2 kernel
def _build_phase2(reps=1, use_ag=True):
    """Column-sharded output GEMM with the head-mixer folded into Wcomb.

    Each core: AllGather full y_att^T [4096, 2048] from per-core slices,
    then out^T[my 512 cols, :] = Wcomb[:, my cols]^T @ y_att^T.
    """
    nc = bass.Bass(trn_type="TRN2")
    ys_d = nc.dram_tensor("ys", [CPB, T], BF16, kind="ExternalInput")
    wc_d = nc.dram_tensor("wc", [4096, 512], BF16, kind="ExternalInput")
    out_d = nc.dram_tensor("out", [CPB, T], F32, kind="ExternalOutput")

    with tile.TileContext(nc) as tc:
        with tc.tile_pool(name="dram", bufs=1, space="DRAM") as dram, \
             tc.tile_pool(name="wres", bufs=1) as wp, \
             tc.tile_pool(name="ych", bufs=2) as yp, \
             tc.tile_pool(name="accs", bufs=1) as ap_:

            yin = dram.tile([CPB, T], BF16, name="yin")
            ya = [dram.tile([NC, 128, T], BF16, addr_space="Shared" if use_ag else "Local",
                            name=f"ya{i}") for i in range(4)]
            for i in range(4):
                ycp = yp.tile([128, T], BF16, tag="ycp", name=f"ycp{i}")
                nc.sync.dma_start(ycp[:], ys_d[i * 128:(i + 1) * 128, :])
                nc.sync.dma_start(yin[i * 128:(i + 1) * 128, :], ycp[:])
            if use_ag:
                for i in range(4):
                    nc.gpsimd.collective_compute(
                        "AllGather", mybir.AluOpType.bypass,
                        replica_groups=[list(range(NC))],
                        ins=[yin[i * 128:(i + 1) * 128, :].opt()],
                        outs=[ya[i][:].opt()])

            # Wcomb columns resident: 32 chan-chunk tiles [128, 512]
            wc_s = [wp.tile([128, 512], BF16, tag=f"wc{cc}", name=f"wc{cc}")
                    for cc in range(32)]
            for cc in range(32):
                nc.sync.dma_start(wc_s[cc][:], wc_d[cc * 128:(cc + 1) * 128, :])

            acc = [ap_.tile([128, T], F32, tag=f"acc{o}", name=f"acc{o}")
                   for o in range(4)]

            for _rep in range(reps):
                with tc.tile_pool(name="gpsum", bufs=4, space="PSUM") as gps:
                    # wave i supplies chan chunks cc = r*4 + i (r = source core)
                    for i in range(4):
                        ych = [yp.tile([128, T], BF16, tag=f"ych{r}", name=f"ych{i}_{r}")
                               for r in range(NC)]
                        for r in range(NC):
                            nc.sync.dma_start(ych[r][:], ya[i][r, :, :])
                        for o in range(4):
                            for tq in range(4):
                                ps = gps.tile([128, TBLK], F32, tag="ps")
                                for r in range(NC):
                                    cc = r * 4 + i
                                    nc.tensor.matmul(
                                        ps[:],
                                        wc_s[cc][:, o * 128:(o + 1) * 128],
                                        ych[r][:, tq * TBLK:(tq + 1) * TBLK],
                                        start=(r == 0), stop=(r == NC - 1))
                                dst = acc[o][:, tq * TBLK:(tq + 1) * TBLK]
                                if i == 0:
                                    nc.scalar.copy(dst, ps[:])
                                else:
                                    nc.vector.tensor_add(dst, dst, ps[:])
                for o in range(4):
                    nc.sync.dma_start(out_d[o * 128:(o + 1) * 128, :], acc[o][:])
    return nc


_cache = {}


def _get_kernels(reps=(1, 1), use_ag=True):
    key = ("p", reps, use_ag)
    if key not in _cache:
        _install_patch()
        _cache[key] = (_build_phase1(reps[0], use_ag),
                       _build_phase2(reps[1], use_ag))
    return _cache[key]


# ------------------------------------------------------------- host wrapper
_host_cache = {}


def _harr(a):
    a = np.asarray(a)
    return (a.shape, str(a.dtype), hash(a.tobytes()[:4096]), hash(a.tobytes()[-4096:]))


def _prep_weights(Wq, Wk, Wv, Wo, mixer_W, mixer_beta, freqs_cos, freqs_sin):
    key = tuple(_harr(a) for a in (Wq, Wk, Wv, Wo, mixer_W, mixer_beta,
                                   freqs_cos, freqs_sin))
    if key in _host_cache:
        return _host_cache[key]

    perm = np.concatenate([np.arange(0, 128, 2), np.arange(1, 128, 2)])
    colperm = np.concatenate([h * 128 + perm for h in range(H)])

    weq = _weff(np.asarray(Wq, np.float32))[:, colperm]
    wek = _weff(np.asarray(Wk, np.float32))[:, colperm]
    wev = _weff(np.asarray(Wv, np.float32))

    def qk_layout(w):  # [4096, 512] -> [512, 4096] strips (colb*128+p, c*128+m)
        return np.ascontiguousarray(
            w.reshape(32, 128, 4, 128).transpose(2, 1, 0, 3).reshape(512, 4096)
        ).astype(BF)

    def v_layout(w):   # [4096, 512] -> [128, 32*512]
        return np.ascontiguousarray(
            w.reshape(32, 128, 512).transpose(1, 0, 2).reshape(128, 32 * 512)
        ).astype(BF)

    wq_l = [qk_layout(weq[:, c * CPB:(c + 1) * CPB]) for c in range(NC)]
    wk_l = [qk_layout(wek[:, c * CPB:(c + 1) * CPB]) for c in range(NC)]
    wv_l = [v_layout(wev[:, c * CPB:(c + 1) * CPB]) for c in range(NC)]

    csT = np.asarray(freqs_cos, np.float32).T               # [64, T]
    snT = np.asarray(freqs_sin, np.float32).T
    cs_host = np.ascontiguousarray(np.concatenate([csT, csT], 0))        # [128,T]
    sn_host = np.ascontiguousarray(np.concatenate([-snT, snT], 0))

    f = np.arange(128)[None, :]
    p = np.arange(128)[:, None]
    mk_host = np.ascontiguousarray((f >= p).astype(np.float32)).astype(BF)

    # ---- fold octonion head-mixer into Wo:  Wcomb[g,:] = M_g^T @ Wo_eff[g,:]
    beta = np.asarray(mixer_beta, np.float32)
    mw = np.asarray(mixer_W, np.float32)
    weo = _weff(np.asarray(Wo, np.float32))                 # [4096, 4096] f32
    wcomb = np.empty((C, C), np.float32)
    Mg = np.empty((1024, 1024), np.float32)
    for g in range(4):
        for i_ in range(8):
            for j_ in range(8):
                Mg[i_ * 128:(i_ + 1) * 128, j_ * 128:(j_ + 1) * 128] = \
                    (_SIGN[i_, j_] * mw[i_ ^ j_].T) * beta[:, None]
        # Mg[(i,e),(j,d)] = S[i,j]*Wm[i^j][d,e]*beta[e]
        wcomb[g * 1024:(g + 1) * 1024, :] = \
            Mg.T @ weo[g * 1024:(g + 1) * 1024, :]
    wc_l = [np.ascontiguousarray(wcomb[:, c * CPB:(c + 1) * CPB]).astype(BF)
            for c in range(NC)]

    res = (wq_l, wk_l, wv_l, wc_l, cs_host, sn_host, mk_host)
    _host_cache.clear()
    _host_cache[key] = res
    return res


def kernel(x, Wq, Wk, Wv, Wo, mixer_W, mixer_beta, freqs_cos, freqs_sin,
           _trace=False, _reps=(1, 1)):
    from concourse.bass_utils import run_bass_kernel_spmd

    x = np.asarray(x, np.float32)
    nc1, nc2 = _get_kernels(_reps)
    wq_l, wk_l, wv_l, wc_l, cs_host, sn_host, mk_host = _prep_weights(
        Wq, Wk, Wv, Wo, mixer_W, mixer_beta, freqs_cos, freqs_sin)

    xT = np.ascontiguousarray(x[0].T).astype(BF)            # [C, T] bf16

    in_maps1 = []
    for c in range(NC):
        in_maps1.append(dict(
            xs=np.ascontiguousarray(xT[c * CPB:(c + 1) * CPB, :]),
            wq=wq_l[c], wk=wk_l[c], wv=wv_l[c],
            cs=cs_host, sn=sn_host, mk=mk_host,
        ))

    r1 = run_bass_kernel_spmd(nc1, in_maps1, core_ids=list(range(NC)),
                              trace=_trace)

    in_maps2 = []
    for c in range(NC):
        in_maps2.append(dict(
            ys=np.ascontiguousarray(np.asarray(r1.results[c]["yt"], BF)),
            wc=wc_l[c],
        ))

    r2 = run_bass_kernel_spmd(nc2, in_maps2, core_ids=list(range(NC)),
                              trace=_trace)
    # out^T blocks [512, 2048] per core -> out [T, C]
    outT = np.concatenate([r2.results[c]["out"] for c in range(NC)], 0)  # [C, T]
    out = np.ascontiguousarray(outT.T).reshape(1, T, C).astype(np.float32)
    return (out, (r1, r2)) if _trace else out
